# revision 23
# baseline (speedup 1.0000x reference)
import sys

sys.path.insert(0, "/opt/trn_rl_repo")
import math

import ml_dtypes
import numpy as np

import concourse.bacc as bacc
import concourse.mybir as mybir
from concourse.tile import TileContext
from concourse.bass_utils import run_bass_kernel_spmd

B = 4096
OBS = 4096
ENC = 1024
LAT = 512
ACT = 3
NMAX = 8
NCORES = 8
BC = B // NCORES  # 512 batch rows per core

F32 = mybir.dt.float32
F32R = mybir.dt.float32r
BF16 = mybir.dt.bfloat16
AF = mybir.ActivationFunctionType
ALU = mybir.AluOpType

_CACHE = {}


def _build(sched):
    # sched: tuple of (f_t, g_t) per active GRU step; f = matmul free dim,
    # g = #columns whose h actually updates (sorted-desc by neighbor count)
    key = ("nc", sched)
    if key in _CACHE:
        return _CACHE[key]
    nc = bacc.Bacc("TRN2", target_bir_lowering=False, debug=False, num_devices=NCORES)

    obsT = nc.dram_tensor("obsT", [OBS, BC], BF16, kind="ExternalInput")
    xT = nc.dram_tensor("xT", [NMAX * LAT, BC], BF16, kind="ExternalInput")
    maskrep = nc.dram_tensor("maskrep", [128, NMAX * BC], F32, kind="ExternalInput")
    wdsT = nc.dram_tensor("wdsT", [OBS, ENC], BF16, kind="ExternalInput")
    wpolT = nc.dram_tensor("wpolT", [ENC, LAT], F32R, kind="ExternalInput")
    wvT = nc.dram_tensor("wvT", [ENC, 1], F32R, kind="ExternalInput")
    wihT = nc.dram_tensor("wihT", [LAT, 3 * LAT], BF16, kind="ExternalInput")
    whhT = nc.dram_tensor("whhT", [LAT, 3 * LAT], BF16, kind="ExternalInput")
    wprT = nc.dram_tensor("wprT", [LAT, ACT], BF16, kind="ExternalInput")
    bds = nc.dram_tensor("bds", [128, 8], F32, kind="ExternalInput")
    bpol = nc.dram_tensor("bpol", [128, 4], F32, kind="ExternalInput")
    bv = nc.dram_tensor("bv", [1, 1], F32, kind="ExternalInput")
    brf = nc.dram_tensor("brf", [128, 4], F32, kind="ExternalInput")
    bzn = nc.dram_tensor("bzn", [128, 4], F32, kind="ExternalInput")
    bin_ = nc.dram_tensor("bin", [128, 4], F32, kind="ExternalInput")
    bhn = nc.dram_tensor("bhn", [128, 4], F32, kind="ExternalInput")
    bpr = nc.dram_tensor("bpr", [ACT, 1], F32, kind="ExternalInput")
    logitsT = nc.dram_tensor("logitsT", [ACT, BC], F32, kind="ExternalOutput")
    svT = nc.dram_tensor("svT", [1, BC], F32, kind="ExternalOutput")

    def mm(out, lhsT, rhs, start, stop):
        nc.tensor.matmul(out, lhsT, rhs, start=start, stop=stop)

    with TileContext(nc) as tc:
        with (
            tc.tile_pool(name="const", bufs=1) as cpool,
            tc.tile_pool(name="wbulk", bufs=1) as wpool,
            tc.tile_pool(name="enc", bufs=1) as epool,
            tc.tile_pool(name="stream", bufs=4) as spool,
            tc.tile_pool(name="x", bufs=4) as xpool,
            tc.tile_pool(name="h", bufs=3) as hpool,
            tc.tile_pool(name="gate", bufs=3) as gpool,
            tc.tile_pool(name="out", bufs=1) as opool,
        ):
            # small consts on the scalar queue; sync queue is the enc stream
            # and gpsimd is reserved for the GRU xk stream
            bds_t = cpool.tile([128, 8], F32)
            nc.scalar.dma_start(bds_t[:], bds[:])
            bpol_t = cpool.tile([128, 4], F32)
            nc.scalar.dma_start(bpol_t[:], bpol[:])
            bv_t = cpool.tile([1, 1], F32)
            nc.scalar.dma_start(bv_t[:], bv[:])
            brf_t = cpool.tile([128, 4], F32)
            nc.scalar.dma_start(brf_t[:], brf[:])
            bzn_t = cpool.tile([128, 4], F32)
            nc.scalar.dma_start(bzn_t[:], bzn[:])
            bin_t = cpool.tile([128, 4], F32)
            nc.scalar.dma_start(bin_t[:], bin_[:])
            bhn_t = cpool.tile([128, 4], F32)
            nc.scalar.dma_start(bhn_t[:], bhn[:])
            bpr_t = cpool.tile([ACT, 1], F32)
            nc.scalar.dma_start(bpr_t[:], bpr[:])

            # GRU weights prefetch on the scalar queue, issued before the enc
            # ACTs occupy that engine: they are needed right when pol ends,
            # before the sync queue would drain them
            wih_t = [wpool.tile([128, 3 * LAT], BF16, name=f"wih{k}") for k in range(4)]
            whh_t = [wpool.tile([128, 3 * LAT], BF16, name=f"whh{k}") for k in range(4)]
            for k in range(4):
                nc.scalar.dma_start(wih_t[k][:], wihT[k * 128 : (k + 1) * 128, :])
                nc.scalar.dma_start(whh_t[k][:], whhT[k * 128 : (k + 1) * 128, :])
            wpr_t = [wpool.tile([128, ACT], BF16, name=f"wpr{k}") for k in range(4)]
            for k in range(4):
                nc.scalar.dma_start(wpr_t[k][:], wprT[k * 128 : (k + 1) * 128, :])

            # ---------------- enc = relu(W_ds @ obs.T + b_ds) ----------------
            encT = [epool.tile([128, BC], F32R, name=f"encT{m}") for m in range(8)]
            with tc.tile_pool(name="penc", bufs=1, space="PSUM") as pe_pool:
                psum_e = [pe_pool.tile([128, BC], F32, name=f"psum_e{m}") for m in range(8)]
                KT = OBS // 128  # 32
                for k in range(KT):
                    wt = spool.tile([128, ENC], BF16, tag="wds")
                    nc.sync.dma_start(wt[:], wdsT[k * 128 : (k + 1) * 128, :])
                    ot = spool.tile([128, BC], BF16, tag="obs")
                    nc.sync.dma_start(ot[:], obsT[k * 128 : (k + 1) * 128, :])
                    for m in range(8):
                        mm(psum_e[m][:], wt[:, m * 128 : (m + 1) * 128], ot[:],
                           start=(k == 0), stop=(k == KT - 1))
                for m in range(8):
                    nc.scalar.activation(encT[m][:], psum_e[m][:], AF.Relu,
                                         bias=bds_t[:, m : m + 1])

            # all bulk weights on the sync queue: serialized behind the enc
            # stream in order, so nothing races it for DMA bandwidth
            wpol_t = [wpool.tile([128, LAT], F32R, name=f"wpol{k}") for k in range(8)]
            for k in range(8):
                nc.sync.dma_start(wpol_t[k][:], wpolT[k * 128 : (k + 1) * 128, :])
            wv_t = [wpool.tile([128, 1], F32R, name=f"wv{k}") for k in range(8)]
            for k in range(8):
                nc.sync.dma_start(wv_t[k][:], wvT[k * 128 : (k + 1) * 128, :])
            mask_t = cpool.tile([128, NMAX * BC], F32)
            nc.sync.dma_start(mask_t[:], maskrep[:])

            # ---------------- pol (h0) and state value ----------------
            hcur = []
            with tc.tile_pool(name="ppol", bufs=1, space="PSUM") as pp_pool:
                psum_p = [pp_pool.tile([128, BC], F32, name=f"psum_p{m}") for m in range(4)]
                psum_v = pp_pool.tile([1, BC], F32)
                for m in range(4):
                    for k in range(8):
                        mm(psum_p[m][:], wpol_t[k][:, m * 128 : (m + 1) * 128],
                           encT[k][:], start=(k == 0), stop=(k == 7))
                for k in range(8):
                    mm(psum_v[:], wv_t[k][:], encT[k][:], start=(k == 0), stop=(k == 7))
                for l in range(4):
                    h0 = hpool.tile([128, BC], BF16, tag=f"h{l}")
                    nc.scalar.activation(h0[:], psum_p[l][:], AF.Relu,
                                         bias=bpol_t[:, l : l + 1])
                    hcur.append(h0)
                sv_t = opool.tile([1, BC], F32)
                nc.scalar.activation(sv_t[:], psum_v[:], AF.Tanh, bias=bv_t[0:1, 0:1])
                nc.sync.dma_start(svT[:], sv_t[:])

            # ---------------- GRU: masked steps, free dim truncated ----------
            with tc.tile_pool(name="pgru", bufs=2, space="PSUM") as pg_pool:
                for t, (f, g) in enumerate(sched):
                    xk = []
                    for k in range(4):
                        xt = xpool.tile([128, BC], BF16, tag=f"x{k}")
                        nc.gpsimd.dma_start(
                            xt[:, 0:f],
                            xT[t * LAT + k * 128 : t * LAT + (k + 1) * 128, 0:f])
                        xk.append(xt)
                    hnew = []
                    for l in range(4):
                        co = l * 128
                        pr = pg_pool.tile([128, BC], F32, tag="pr")
                        pz = pg_pool.tile([128, BC], F32, tag="pz")
                        pgi = pg_pool.tile([128, BC], F32, tag="pgi")
                        pgh = pg_pool.tile([128, BC], F32, tag="pgh")
                        # x-side (independent of h)
                        for k in range(4):
                            mm(pr[:, 0:f], wih_t[k][:, co : co + 128], xk[k][:, 0:f],
                               start=(k == 0), stop=False)
                        for k in range(4):
                            mm(pz[:, 0:f], wih_t[k][:, LAT + co : LAT + co + 128],
                               xk[k][:, 0:f], start=(k == 0), stop=False)
                        for k in range(4):
                            mm(pgi[:, 0:f], wih_t[k][:, 2 * LAT + co : 2 * LAT + co + 128],
                               xk[k][:, 0:f], start=(k == 0), stop=(k == 3))
                        # h-side
                        for k in range(4):
                            mm(pgh[:, 0:f], whh_t[k][:, 2 * LAT + co : 2 * LAT + co + 128],
                               hcur[k][:, 0:f], start=(k == 0), stop=(k == 3))
                        for k in range(4):
                            mm(pr[:, 0:f], whh_t[k][:, co : co + 128], hcur[k][:, 0:f],
                               start=False, stop=(k == 3))
                        for k in range(4):
                            mm(pz[:, 0:f], whh_t[k][:, LAT + co : LAT + co + 128],
                               hcur[k][:, 0:f], start=False, stop=(k == 3))
                        # gates, on the g active columns only
                        gb = gpool.tile([128, BC], F32, tag="gb")
                        nc.scalar.activation(gb[:, 0:g], pgh[:, 0:g], AF.Identity,
                                             bias=bhn_t[:, l : l + 1])
                        r = gpool.tile([128, BC], F32, tag="r")
                        nc.scalar.activation(r[:, 0:g], pr[:, 0:g], AF.Sigmoid,
                                             bias=brf_t[:, l : l + 1])
                        nc.vector.tensor_mul(gb[:, 0:g], r[:, 0:g], gb[:, 0:g])
                        # zc = 1 - z = sigmoid(-(pz + b_z)); bzn = -b_z
                        zc = gpool.tile([128, BC], F32, tag="zc")
                        nc.scalar.activation(zc[:, 0:g], pz[:, 0:g], AF.Sigmoid,
                                             bias=bzn_t[:, l : l + 1], scale=-1.0)
                        # pre_n = (pgi + b_in) + r*(gh_n + b_hn)
                        nc.vector.scalar_tensor_tensor(
                            gb[:, 0:g], pgi[:, 0:g], bin_t[:, l : l + 1], gb[:, 0:g],
                            op0=ALU.add, op1=ALU.add)
                        n = gpool.tile([128, BC], F32, tag="n")
                        nc.scalar.activation(n[:, 0:g], gb[:, 0:g], AF.Tanh)
                        nc.vector.tensor_mul(zc[:, 0:g], zc[:, 0:g],
                                             mask_t[:, t * BC : t * BC + g])
                        nc.vector.tensor_sub(n[:, 0:g], n[:, 0:g], hcur[l][:, 0:g])
                        nc.vector.tensor_mul(n[:, 0:g], zc[:, 0:g], n[:, 0:g])
                        hn = hpool.tile([128, BC], BF16, tag=f"h{l}")
                        nc.vector.tensor_add(hn[:, 0:g], hcur[l][:, 0:g], n[:, 0:g])
                        if g < BC:
                            nc.vector.tensor_copy(hn[:, g:BC], hcur[l][:, g:BC])
                        hnew.append(hn)
                    hcur = hnew

            # ---------------- logits = relu(W_pr @ h + b_pr); softmax on host
            with tc.tile_pool(name="pout", bufs=1, space="PSUM") as po_pool:
                ppr = po_pool.tile([ACT, BC], F32)
                for k in range(4):
                    mm(ppr[:], wpr_t[k][:], hcur[k][:], start=(k == 0), stop=(k == 3))
                logits = opool.tile([ACT, BC], F32)
                nc.scalar.activation(logits[:], ppr[:], AF.Relu, bias=bpr_t[0:ACT, 0:1])
                nc.sync.dma_start(logitsT[:], logits[:])

    nc.compile()
    _CACHE[key] = nc
    _CACHE["nc"] = nc
    return nc


def _schedule(cnt_sorted_desc):
    # global counts sorted desc; core r col j holds global rank j*8+r
    sched = []
    for t in range(NMAX):
        G = int((cnt_sorted_desc > t).sum())
        if G == 0:
            break
        n = math.ceil(G / NCORES)
        g = min(BC, 32 * math.ceil(n / 32))
        f = g  # bf16 matmul is 1 cyc/row at any free size
        sched.append((f, g))
    return tuple(sched)


def _prep_core(rows, observation, neighbors, cnt, weights):
    obsT = np.ascontiguousarray(observation[rows, :].T).astype(ml_dtypes.bfloat16)
    xT = np.ascontiguousarray(
        neighbors[rows].transpose(1, 2, 0).reshape(NMAX * LAT, BC)
    ).astype(ml_dtypes.bfloat16)
    m = (np.arange(NMAX, dtype=np.int32)[:, None] < cnt[None, rows]).astype(np.float32)
    maskrep = np.ascontiguousarray(
        np.broadcast_to(m.reshape(1, NMAX, BC), (128, NMAX, BC)).reshape(128, NMAX * BC))
    d = {"obsT": obsT, "xT": xT, "maskrep": maskrep}
    d.update(weights)
    return d


def kernel(**inputs):
    inputs = {k: np.asarray(v) for k, v in inputs.items()}
    observation = inputs["observation"].astype(np.float32, copy=False)
    neighbors = inputs["neighbors"].astype(np.float32, copy=False)
    cnt = inputs["neighbor_counts"]
    f32 = lambda a: np.ascontiguousarray(a.astype(np.float32, copy=False))
    W_ds, b_ds = inputs["W_ds"], inputs["b_ds"]
    W_pol, b_pol = inputs["W_pol"], inputs["b_pol"]
    W_v, b_v = inputs["W_v"], inputs["b_v"]
    W_ih, b_ih = inputs["W_ih"], inputs["b_ih"]
    W_hh, b_hh = inputs["W_hh"], inputs["b_hh"]
    W_pr, b_pr = inputs["W_pr"], inputs["b_pr"]

    weights = {
        "wdsT": np.ascontiguousarray(W_ds.T).astype(ml_dtypes.bfloat16),
        "wpolT": f32(W_pol.T),
        "wvT": f32(W_v.T),
        "wihT": np.ascontiguousarray(W_ih.T).astype(ml_dtypes.bfloat16),
        "whhT": np.ascontiguousarray(W_hh.T).astype(ml_dtypes.bfloat16),
        "wprT": np.ascontiguousarray(W_pr.T).astype(ml_dtypes.bfloat16),
        "bds": f32(b_ds.reshape(8, 128).T),
        "bpol": f32(b_pol.reshape(4, 128).T),
        "bv": f32(b_v.reshape(1, 1)),
        "brf": f32((b_ih[0:LAT] + b_hh[0:LAT]).reshape(4, 128).T),
        "bzn": f32(-(b_ih[LAT : 2 * LAT] + b_hh[LAT : 2 * LAT]).reshape(4, 128).T),
        "bin": f32(b_ih[2 * LAT : 3 * LAT].reshape(4, 128).T),
        "bhn": f32(b_hh[2 * LAT : 3 * LAT].reshape(4, 128).T),
        "bpr": f32(b_pr.reshape(ACT, 1)),
    }

    # sort batch rows by neighbor count desc; deal round-robin so every core
    # sees the same per-column count profile -> shared truncation schedule
    order = np.argsort(-cnt.astype(np.int64), kind="stable")
    sched = _schedule(cnt[order])
    core_rows = [order[r::NCORES] for r in range(NCORES)]

    _CACHE["weights"] = weights
    nc = _build(sched)
    in_maps = [_prep_core(core_rows[r], observation, neighbors, cnt, weights)
               for r in range(NCORES)]
    _CACHE["in_maps"] = in_maps
    res = run_bass_kernel_spmd(nc, in_maps, list(range(NCORES)))

    logits = np.empty((B, ACT), dtype=np.float64)
    state_vals = np.empty((B, 1), dtype=np.float32)
    for r in range(NCORES):
        logits[core_rows[r], :] = res.results[r]["logitsT"].T
        state_vals[core_rows[r], :] = res.results[r]["svT"].T
    e = np.exp(logits - logits.max(axis=1, keepdims=True))
    probs = (e / e.sum(axis=1, keepdims=True)).astype(np.float32)
    return probs, state_vals


# revision 29
# speedup vs baseline: 1.2329x; 1.2329x over previous
import sys

sys.path.insert(0, "/opt/trn_rl_repo")
import math

import ml_dtypes
import numpy as np

import concourse.bacc as bacc
import concourse.mybir as mybir
from concourse.tile import TileContext
from concourse.bass_utils import run_bass_kernel_spmd

B = 4096
OBS = 4096
ENC = 1024
LAT = 512
ACT = 3
NMAX = 8
NCORES = 8
BC = B // NCORES  # 512 batch rows per core

F32 = mybir.dt.float32
F32R = mybir.dt.float32r
BF16 = mybir.dt.bfloat16
AF = mybir.ActivationFunctionType
ALU = mybir.AluOpType

_CACHE = {}


def _build(sched):
    # sched: tuple of (f_t, g_t) per active GRU step; f = matmul free dim,
    # g = #columns whose h actually updates (sorted-desc by neighbor count)
    key = ("nc", sched)
    if key in _CACHE:
        return _CACHE[key]
    nc = bacc.Bacc("TRN2", target_bir_lowering=False, debug=False, num_devices=NCORES)

    obsT = nc.dram_tensor("obsT", [OBS, BC], BF16, kind="ExternalInput")
    xT = nc.dram_tensor("xT", [NMAX * LAT, BC], BF16, kind="ExternalInput")
    maskrep = nc.dram_tensor("maskrep", [128, NMAX * BC], BF16, kind="ExternalInput")
    wdsT = nc.dram_tensor("wdsT", [OBS, ENC], BF16, kind="ExternalInput")
    wpolT = nc.dram_tensor("wpolT", [ENC, LAT], BF16, kind="ExternalInput")
    wvT = nc.dram_tensor("wvT", [ENC, 1], BF16, kind="ExternalInput")
    wihT = nc.dram_tensor("wihT", [LAT, 3 * LAT], BF16, kind="ExternalInput")
    whhT = nc.dram_tensor("whhT", [LAT, 3 * LAT], BF16, kind="ExternalInput")
    wprT = nc.dram_tensor("wprT", [LAT, ACT], BF16, kind="ExternalInput")
    bds = nc.dram_tensor("bds", [128, 8], F32, kind="ExternalInput")
    bpol = nc.dram_tensor("bpol", [128, 4], F32, kind="ExternalInput")
    bv = nc.dram_tensor("bv", [1, 1], F32, kind="ExternalInput")
    brf = nc.dram_tensor("brf", [128, 4], F32, kind="ExternalInput")
    bzn = nc.dram_tensor("bzn", [128, 4], F32, kind="ExternalInput")
    bin_ = nc.dram_tensor("bin", [128, 4], F32, kind="ExternalInput")
    bhn = nc.dram_tensor("bhn", [128, 4], F32, kind="ExternalInput")
    bpr = nc.dram_tensor("bpr", [ACT, 1], F32, kind="ExternalInput")
    logitsT = nc.dram_tensor("logitsT", [ACT, BC], F32, kind="ExternalOutput")
    svT = nc.dram_tensor("svT", [1, BC], F32, kind="ExternalOutput")

    def mm(out, lhsT, rhs, start, stop):
        nc.tensor.matmul(out, lhsT, rhs, start=start, stop=stop)

    with TileContext(nc) as tc:
        with (
            tc.tile_pool(name="const", bufs=1) as cpool,
            tc.tile_pool(name="wbulk", bufs=1) as wpool,
            tc.tile_pool(name="enc", bufs=1) as epool,
            tc.tile_pool(name="stream", bufs=4) as spool,
            tc.tile_pool(name="x", bufs=2) as xpool,
            tc.tile_pool(name="h", bufs=2) as hpool,
            tc.tile_pool(name="gate", bufs=2) as gpool,
            tc.tile_pool(name="out", bufs=1) as opool,
        ):
            # small consts on the scalar queue; sync queue is the enc stream
            # and gpsimd is reserved for the GRU xk stream
            bds_t = cpool.tile([128, 8], F32)
            nc.scalar.dma_start(bds_t[:], bds[:])
            bpol_t = cpool.tile([128, 4], F32)
            nc.scalar.dma_start(bpol_t[:], bpol[:])
            bv_t = cpool.tile([1, 1], F32)
            nc.scalar.dma_start(bv_t[:], bv[:])
            brf_t = cpool.tile([128, 4], F32)
            nc.scalar.dma_start(brf_t[:], brf[:])
            bzn_t = cpool.tile([128, 4], F32)
            nc.scalar.dma_start(bzn_t[:], bzn[:])
            bin_t = cpool.tile([128, 4], F32)
            nc.scalar.dma_start(bin_t[:], bin_[:])
            bhn_t = cpool.tile([128, 4], F32)
            nc.scalar.dma_start(bhn_t[:], bhn[:])
            bpr_t = cpool.tile([ACT, 1], F32)
            nc.scalar.dma_start(bpr_t[:], bpr[:])

            # ---------------- enc = relu(W_ds @ obs.T + b_ds) ----------------
            encT = [epool.tile([128, BC], BF16, name=f"encT{m}") for m in range(8)]
            with tc.tile_pool(name="penc", bufs=1, space="PSUM") as pe_pool:
                psum_e = [pe_pool.tile([128, BC], F32, name=f"psum_e{m}") for m in range(8)]
                KT = OBS // 128  # 32
                for k in range(KT):
                    wt = spool.tile([128, ENC], BF16, tag="wds")
                    nc.sync.dma_start(wt[:], wdsT[k * 128 : (k + 1) * 128, :])
                    ot = spool.tile([128, BC], BF16, tag="obs")
                    nc.sync.dma_start(ot[:], obsT[k * 128 : (k + 1) * 128, :])
                    for m in range(8):
                        mm(psum_e[m][:], wt[:, m * 128 : (m + 1) * 128], ot[:],
                           start=(k == 0), stop=(k == KT - 1))
                for m in range(8):
                    nc.scalar.activation(encT[m][:], psum_e[m][:], AF.Relu,
                                         bias=bds_t[:, m : m + 1])

            # all bulk weights on the sync queue: serialized behind the enc
            # stream in order, so nothing races it for DMA bandwidth
            wpol_t = [wpool.tile([128, LAT], BF16, name=f"wpol{k}") for k in range(8)]
            for k in range(8):
                nc.sync.dma_start(wpol_t[k][:], wpolT[k * 128 : (k + 1) * 128, :])
            wv_t = [wpool.tile([128, 1], BF16, name=f"wv{k}") for k in range(8)]
            for k in range(8):
                nc.sync.dma_start(wv_t[k][:], wvT[k * 128 : (k + 1) * 128, :])
            wih_t = [wpool.tile([128, 3 * LAT], BF16, name=f"wih{k}") for k in range(4)]
            whh_t = [wpool.tile([128, 3 * LAT], BF16, name=f"whh{k}") for k in range(4)]
            for k in range(4):
                nc.sync.dma_start(wih_t[k][:], wihT[k * 128 : (k + 1) * 128, :])
                nc.sync.dma_start(whh_t[k][:], whhT[k * 128 : (k + 1) * 128, :])
            mask_t = cpool.tile([128, NMAX * BC], BF16)
            nc.sync.dma_start(mask_t[:], maskrep[:])
            # wpr last: only needed for the final logits matmul
            wpr_t = [wpool.tile([128, ACT], BF16, name=f"wpr{k}") for k in range(4)]
            for k in range(4):
                nc.sync.dma_start(wpr_t[k][:], wprT[k * 128 : (k + 1) * 128, :])

            # ---------------- pol (h0) and state value ----------------
            hcur = []
            with tc.tile_pool(name="ppol", bufs=1, space="PSUM") as pp_pool:
                psum_p = [pp_pool.tile([128, BC], F32, name=f"psum_p{m}") for m in range(4)]
                psum_v = pp_pool.tile([1, BC], F32)
                for m in range(4):
                    for k in range(8):
                        mm(psum_p[m][:], wpol_t[k][:, m * 128 : (m + 1) * 128],
                           encT[k][:], start=(k == 0), stop=(k == 7))
                for k in range(8):
                    mm(psum_v[:], wv_t[k][:], encT[k][:], start=(k == 0), stop=(k == 7))
                for l in range(4):
                    h0 = hpool.tile([128, BC], BF16, tag=f"h{l}")
                    nc.scalar.activation(h0[:], psum_p[l][:], AF.Relu,
                                         bias=bpol_t[:, l : l + 1])
                    hcur.append(h0)
                sv_t = opool.tile([1, BC], F32)
                nc.scalar.activation(sv_t[:], psum_v[:], AF.Tanh, bias=bv_t[0:1, 0:1])
                nc.sync.dma_start(svT[:], sv_t[:])

            # ---------------- GRU: masked steps, free dim truncated ----------
            with tc.tile_pool(name="pgru", bufs=2, space="PSUM") as pg_pool:
                for t, (f, g) in enumerate(sched):
                    xk = []
                    for k in range(4):
                        xt = xpool.tile([128, BC], BF16, tag=f"x{k}")
                        nc.gpsimd.dma_start(
                            xt[:, 0:f],
                            xT[t * LAT + k * 128 : t * LAT + (k + 1) * 128, 0:f])
                        xk.append(xt)
                    hnew = []
                    for l in range(4):
                        co = l * 128
                        pr = pg_pool.tile([128, BC], F32, tag="pr")
                        pz = pg_pool.tile([128, BC], F32, tag="pz")
                        pgi = pg_pool.tile([128, BC], F32, tag="pgi")
                        pgh = pg_pool.tile([128, BC], F32, tag="pgh")
                        # x-side (independent of h)
                        for k in range(4):
                            mm(pr[:, 0:f], wih_t[k][:, co : co + 128], xk[k][:, 0:f],
                               start=(k == 0), stop=False)
                        for k in range(4):
                            mm(pz[:, 0:f], wih_t[k][:, LAT + co : LAT + co + 128],
                               xk[k][:, 0:f], start=(k == 0), stop=False)
                        for k in range(4):
                            mm(pgi[:, 0:f], wih_t[k][:, 2 * LAT + co : 2 * LAT + co + 128],
                               xk[k][:, 0:f], start=(k == 0), stop=(k == 3))
                        # h-side
                        for k in range(4):
                            mm(pgh[:, 0:f], whh_t[k][:, 2 * LAT + co : 2 * LAT + co + 128],
                               hcur[k][:, 0:f], start=(k == 0), stop=(k == 3))
                        for k in range(4):
                            mm(pr[:, 0:f], whh_t[k][:, co : co + 128], hcur[k][:, 0:f],
                               start=False, stop=(k == 3))
                        for k in range(4):
                            mm(pz[:, 0:f], whh_t[k][:, LAT + co : LAT + co + 128],
                               hcur[k][:, 0:f], start=False, stop=(k == 3))
                        # gates, on the g active columns only
                        gb = gpool.tile([128, BC], F32, tag="gb")
                        nc.scalar.activation(gb[:, 0:g], pgh[:, 0:g], AF.Identity,
                                             bias=bhn_t[:, l : l + 1])
                        r = gpool.tile([128, BC], F32, tag="r")
                        nc.scalar.activation(r[:, 0:g], pr[:, 0:g], AF.Sigmoid,
                                             bias=brf_t[:, l : l + 1])
                        nc.vector.tensor_mul(gb[:, 0:g], r[:, 0:g], gb[:, 0:g])
                        # zc = 1 - z = sigmoid(-(pz + b_z)); bzn = -b_z
                        zc = gpool.tile([128, BC], F32, tag="zc")
                        nc.scalar.activation(zc[:, 0:g], pz[:, 0:g], AF.Sigmoid,
                                             bias=bzn_t[:, l : l + 1], scale=-1.0)
                        # pre_n = (pgi + b_in) + r*(gh_n + b_hn)
                        nc.vector.scalar_tensor_tensor(
                            gb[:, 0:g], pgi[:, 0:g], bin_t[:, l : l + 1], gb[:, 0:g],
                            op0=ALU.add, op1=ALU.add)
                        n = gpool.tile([128, BC], F32, tag="n")
                        nc.scalar.activation(n[:, 0:g], gb[:, 0:g], AF.Tanh)
                        nc.vector.tensor_mul(zc[:, 0:g], zc[:, 0:g],
                                             mask_t[:, t * BC : t * BC + g])
                        nc.vector.tensor_sub(n[:, 0:g], n[:, 0:g], hcur[l][:, 0:g])
                        nc.vector.tensor_mul(n[:, 0:g], zc[:, 0:g], n[:, 0:g])
                        hn = hpool.tile([128, BC], BF16, tag=f"h{l}")
                        nc.vector.tensor_add(hn[:, 0:g], hcur[l][:, 0:g], n[:, 0:g])
                        if g < BC:
                            nc.vector.tensor_copy(hn[:, g:BC], hcur[l][:, g:BC])
                        hnew.append(hn)
                    hcur = hnew

            # ---------------- logits = relu(W_pr @ h + b_pr); softmax on host
            with tc.tile_pool(name="pout", bufs=1, space="PSUM") as po_pool:
                ppr = po_pool.tile([ACT, BC], F32)
                for k in range(4):
                    mm(ppr[:], wpr_t[k][:], hcur[k][:], start=(k == 0), stop=(k == 3))
                logits = opool.tile([ACT, BC], F32)
                nc.scalar.activation(logits[:], ppr[:], AF.Relu, bias=bpr_t[0:ACT, 0:1])
                nc.sync.dma_start(logitsT[:], logits[:])

    nc.compile()
    _CACHE[key] = nc
    _CACHE["nc"] = nc
    return nc


def _schedule(cnt_sorted_desc):
    # global counts sorted desc; core r col j holds global rank j*8+r
    sched = []
    for t in range(NMAX):
        G = int((cnt_sorted_desc > t).sum())
        if G == 0:
            break
        n = math.ceil(G / NCORES)
        g = min(BC, 32 * math.ceil(n / 32))
        f = g  # bf16 matmul is 1 cyc/row at any free size
        sched.append((f, g))
    return tuple(sched)


def _prep_core(rows, observation, neighbors, cnt, weights):
    obsT = np.ascontiguousarray(observation[rows, :].T).astype(ml_dtypes.bfloat16)
    xT = np.ascontiguousarray(
        neighbors[rows].transpose(1, 2, 0).reshape(NMAX * LAT, BC)
    ).astype(ml_dtypes.bfloat16)
    m = (np.arange(NMAX, dtype=np.int32)[:, None] < cnt[None, rows]).astype(np.float32)
    maskrep = np.ascontiguousarray(
        np.broadcast_to(m.reshape(1, NMAX, BC), (128, NMAX, BC)).reshape(128, NMAX * BC)
    ).astype(ml_dtypes.bfloat16)
    d = {"obsT": obsT, "xT": xT, "maskrep": maskrep}
    d.update(weights)
    return d


def kernel(**inputs):
    inputs = {k: np.asarray(v) for k, v in inputs.items()}
    observation = inputs["observation"].astype(np.float32, copy=False)
    neighbors = inputs["neighbors"].astype(np.float32, copy=False)
    cnt = inputs["neighbor_counts"]
    f32 = lambda a: np.ascontiguousarray(a.astype(np.float32, copy=False))
    W_ds, b_ds = inputs["W_ds"], inputs["b_ds"]
    W_pol, b_pol = inputs["W_pol"], inputs["b_pol"]
    W_v, b_v = inputs["W_v"], inputs["b_v"]
    W_ih, b_ih = inputs["W_ih"], inputs["b_ih"]
    W_hh, b_hh = inputs["W_hh"], inputs["b_hh"]
    W_pr, b_pr = inputs["W_pr"], inputs["b_pr"]

    weights = {
        "wdsT": np.ascontiguousarray(W_ds.T).astype(ml_dtypes.bfloat16),
        "wpolT": np.ascontiguousarray(W_pol.T).astype(ml_dtypes.bfloat16),
        "wvT": np.ascontiguousarray(W_v.T).astype(ml_dtypes.bfloat16),
        "wihT": np.ascontiguousarray(W_ih.T).astype(ml_dtypes.bfloat16),
        "whhT": np.ascontiguousarray(W_hh.T).astype(ml_dtypes.bfloat16),
        "wprT": np.ascontiguousarray(W_pr.T).astype(ml_dtypes.bfloat16),
        "bds": f32(b_ds.reshape(8, 128).T),
        "bpol": f32(b_pol.reshape(4, 128).T),
        "bv": f32(b_v.reshape(1, 1)),
        "brf": f32((b_ih[0:LAT] + b_hh[0:LAT]).reshape(4, 128).T),
        "bzn": f32(-(b_ih[LAT : 2 * LAT] + b_hh[LAT : 2 * LAT]).reshape(4, 128).T),
        "bin": f32(b_ih[2 * LAT : 3 * LAT].reshape(4, 128).T),
        "bhn": f32(b_hh[2 * LAT : 3 * LAT].reshape(4, 128).T),
        "bpr": f32(b_pr.reshape(ACT, 1)),
    }

    # sort batch rows by neighbor count desc; deal round-robin so every core
    # sees the same per-column count profile -> shared truncation schedule
    order = np.argsort(-cnt.astype(np.int64), kind="stable")
    sched = _schedule(cnt[order])
    core_rows = [order[r::NCORES] for r in range(NCORES)]

    _CACHE["weights"] = weights
    nc = _build(sched)
    in_maps = [_prep_core(core_rows[r], observation, neighbors, cnt, weights)
               for r in range(NCORES)]
    _CACHE["in_maps"] = in_maps
    res = run_bass_kernel_spmd(nc, in_maps, list(range(NCORES)))

    logits = np.empty((B, ACT), dtype=np.float64)
    state_vals = np.empty((B, 1), dtype=np.float32)
    for r in range(NCORES):
        logits[core_rows[r], :] = res.results[r]["logitsT"].T
        state_vals[core_rows[r], :] = res.results[r]["svT"].T
    e = np.exp(logits - logits.max(axis=1, keepdims=True))
    probs = (e / e.sum(axis=1, keepdims=True)).astype(np.float32)
    return probs, state_vals
